# revision 12
# baseline (speedup 1.0000x reference)
"""MoE (gating + 8 experts, BN-folded) Trainium2 Bass kernel.

Contract: kernel(**inputs) takes the FULL unsharded inputs (numpy, keyed as in
setup_inputs()) and returns the FULL [65536, 1] float32 output.

Strategy:
  * Data-parallel over 8 NeuronCores: batch 65536 -> 8192 rows per core.
  * All BatchNorms are eval-mode affine maps -> folded into the adjacent
    Linear weights/biases on the host (cheap: params < 2 MB).
  * Activations live on-chip as [features(partitions), batch(free)]; x is
    transposed + pre-tiled host-side per shard so each batch tile is one
    contiguous DMA.
  * All matmuls run bf16 (stationary weights + moving activations).
  * Per 512-row tile the PE stream is phase-ordered (all expert L0 blocks,
    then L1, then L2, interleaved) so every matmul's input activation is
    produced ~2us before the PE reaches it - no PE stalls on Act/DVE.
  * Combine: four zero-padded [128->8] projection matmuls accumulate the
    per-expert output scalars z_e into one PSUM row-block Z[8,TB]; one DVE
    multiply forms gz = exp(logits) * z.  gz and exp(logits) are DMA'd out
    and the host finishes y = sum_e gz / sum_e eg + ob (softmax divide).
  * The combine of tile t is emitted inside tile t+1's matmul stream so its
    matmuls never wait on tile t's last activations.
  * Const DMAs are batched (10 transfers) and ordered by first use so the
    PE starts ~1.5us in and warms up quickly.
"""

import numpy as np
import ml_dtypes

EPS = 1e-5
B, D, E, G, H0, H1, H2 = 65536, 256, 8, 128, 256, 128, 64
NCORES = 8
NB = B // NCORES          # rows per core
TB = 512                  # batch tile (matmul free dim / PSUM bank)
NT = NB // TB             # batch tiles per core
KD = D // 128             # k-chunks over D
NPAIR = E // 2


def _fold_params(inputs):
    """Fold the four BatchNorms into the adjacent Linears. float64 math."""
    f = {k: np.asarray(v, dtype=np.float64) for k, v in inputs.items()}

    s_in = f["in_g"] / np.sqrt(f["in_v"] + EPS)            # [D]
    t_in = f["in_b"] - f["in_m"] * s_in                    # [D]

    # gating L1 (+input BN folded in)
    a_g = f["g_g"] / np.sqrt(f["g_v"] + EPS)               # [G]
    w1 = f["gW1"] * a_g[None, :]                           # [D,G]
    W1f = s_in[:, None] * w1
    b1f = t_in @ w1 + (f["gb1"] - f["g_m"]) * a_g + f["g_b"]

    # expert L0 (+input BN)
    a0 = f["e0g"] / np.sqrt(f["e0v"] + EPS)                # [E,H0]
    w0 = f["eW0"] * a0[:, None, :]                         # [E,D,H0]
    W0f = s_in[None, :, None] * w0
    b0f = np.einsum("d,edo->eo", t_in, w0) + (f["eb0"] - f["e0m"]) * a0 + f["e0b"]

    a1 = f["e1g"] / np.sqrt(f["e1v"] + EPS)
    W1ef = f["eW1"] * a1[:, None, :]                       # [E,H0,H1]
    b1ef = (f["eb1"] - f["e1m"]) * a1 + f["e1b"]

    a2 = f["e2g"] / np.sqrt(f["e2v"] + EPS)
    W2f = f["eW2"] * a2[:, None, :]                        # [E,H1,H2]
    b2f = (f["eb2"] - f["e2m"]) * a2 + f["e2b"]

    bf = lambda a: np.ascontiguousarray(a).astype(ml_dtypes.bfloat16)

    dev = {}
    dev["WG1"] = bf(W1f.reshape(KD, 128, G).transpose(1, 0, 2))          # [128,KD,G]
    dev["WE0"] = bf(W0f.reshape(E, KD, 128, 2, 128).transpose(2, 0, 1, 3, 4))  # [128,E,KD,2,128]
    dev["WE1"] = bf(W1ef.reshape(E, 2, 128, H1).transpose(2, 0, 1, 3))   # [128,E,2,H1]
    # dense [H1,H2] per expert; pair members go to psum partition halves
    # via column-tiled concurrent matmuls
    WE2 = np.zeros((128, NPAIR, 2, H2), dtype=np.float64)
    for j in range(NPAIR):
        WE2[:, j, 0, :] = W2f[2 * j]                       # -> psum parts 0:64
        WE2[:, j, 1, :] = W2f[2 * j + 1]                   # -> psum parts 64:128
    dev["WE2"] = bf(WE2)

    # WGB: [:,0,:] gating L2 weight; [:,1+j,:] zero-padded pair projections
    ow = f["oW"][:, 0]                                     # [H2]
    WGB = np.zeros((128, 1 + NPAIR, E), dtype=np.float64)
    WGB[:, 0, :] = f["gW2"]
    for j in range(NPAIR):
        WGB[0:64, 1 + j, 2 * j] = ow
        WGB[64:128, 1 + j, 2 * j + 1] = ow
    dev["WGB"] = bf(WGB)

    # BIAS blob [128, 8, 4] f32:
    #  [:,e,0] / [:,e,1]  expert L0 bias chunks; [:,e,2] expert L1 bias
    #  [:,j,3] (j<4) paired L2 bias; [:,4,3] gating L1 bias; [0:8,5,3] gating L2 bias
    BIAS = np.zeros((128, E, 4), dtype=np.float64)
    for e in range(E):
        BIAS[:, e, 0] = b0f[e, 0:128]
        BIAS[:, e, 1] = b0f[e, 128:256]
        BIAS[:, e, 2] = b1ef[e]
    for j in range(NPAIR):
        BIAS[0:64, j, 3] = b2f[2 * j]
        BIAS[64:128, j, 3] = b2f[2 * j + 1]
    BIAS[:, 4, 3] = b1f
    BIAS[0:8, 5, 3] = f["gb2"]
    dev["BIAS"] = np.ascontiguousarray(BIAS, dtype=np.float32)

    ob = float(f["ob"][0])
    return dev, ob


def _build_program():
    import concourse.mybir as mybir
    import concourse.tile as tile
    from concourse import bacc

    f32 = mybir.dt.float32
    bf16 = mybir.dt.bfloat16
    Relu = mybir.ActivationFunctionType.Relu
    Exp = mybir.ActivationFunctionType.Exp
    add = mybir.AluOpType.add
    amax = mybir.AluOpType.max

    nc = bacc.Bacc("TRN2", target_bir_lowering=False, debug=False)

    xTd = nc.dram_tensor("xT", [128, NT, KD, TB], bf16, kind="ExternalInput").ap()
    GZd = nc.dram_tensor("GZ", [E, NB], f32, kind="ExternalOutput").ap()
    EGd = nc.dram_tensor("EG", [E, NB], bf16, kind="ExternalOutput").ap()
    d_in = {}
    for name, shape, dt in [
        ("WG1", [128, KD, G], bf16),
        ("WE0", [128, E, KD, 2, 128], bf16),
        ("WE1", [128, E, 2, H1], bf16),
        ("WE2", [128, NPAIR, 2, H2], bf16),
        ("WGB", [128, 1 + NPAIR, E], bf16),
        ("BIAS", [128, E, 4], f32),
    ]:
        d_in[name] = nc.dram_tensor(name, shape, dt, kind="ExternalInput").ap()

    with tile.TileContext(nc) as tc:
        with (
            tc.tile_pool(name="consts", bufs=1) as consts,
            tc.tile_pool(name="sb", bufs=1) as sb,
            tc.tile_pool(name="ps", bufs=1, space="PSUM") as ps,
        ):
            W = {}
            for name, ap in d_in.items():
                W[name] = consts.tile(list(ap.shape), ap.dtype, tag=name, name=name)
            # const DMAs spread across the DMA-capable queues (gpsimd/scalar;
            # sync carries the x tiles) so the initial ~0.7us-each issue ops
            # land in parallel instead of serializing on one queue
            nc.scalar.dma_start(W["WG1"][:], d_in["WG1"][:])
            nc.gpsimd.dma_start(W["WE0"][:, 0:2], d_in["WE0"][:, 0:2])
            nc.scalar.dma_start(W["BIAS"][:], d_in["BIAS"][:])
            nc.scalar.dma_start(W["WGB"][:], d_in["WGB"][:])
            nc.gpsimd.dma_start(W["WE0"][:, 2:4], d_in["WE0"][:, 2:4])
            nc.scalar.dma_start(W["WE0"][:, 4:6], d_in["WE0"][:, 4:6])
            nc.gpsimd.dma_start(W["WE0"][:, 6:8], d_in["WE0"][:, 6:8])
            nc.gpsimd.dma_start(W["WE1"][:, 0:4], d_in["WE1"][:, 0:4])
            nc.gpsimd.dma_start(W["WE1"][:, 4:8], d_in["WE1"][:, 4:8])
            nc.gpsimd.dma_start(W["WE2"][:], d_in["WE2"][:])

            BIAS = W["BIAS"]

            def l0(e, xt):
                h0 = sb.tile([128, 2, TB], bf16, tag="h0", bufs=6, name=f"h0_{e}")
                for mc in (0, 1):
                    p = ps.tile([128, TB], f32, tag="big", bufs=6, name=f"ps0_{e}_{mc}")
                    for c in range(KD):
                        nc.tensor.matmul(p[:], W["WE0"][:, e, c, mc, :], xt[:, c, :],
                                         start=(c == 0), stop=(c == KD - 1))
                    if mc == 0:
                        nc.scalar.activation(h0[:, 0, :], p[:], Relu,
                                             bias=BIAS[:, e, 0:1])
                    else:
                        nc.vector.tensor_scalar(out=h0[:, 1, :], in0=p[:],
                                                scalar1=BIAS[:, e, 1:2], scalar2=0.0,
                                                op0=add, op1=amax)
                return h0

            def l1(e, h0):
                p = ps.tile([128, TB], f32, tag="big", bufs=6, name=f"ps1_{e}")
                for c in (0, 1):
                    nc.tensor.matmul(p[:], W["WE1"][:, e, c, :], h0[:, c, :],
                                     start=(c == 0), stop=(c == 1))
                h1 = sb.tile([128, TB], bf16, tag="h1", bufs=9, name=f"h1_{e}")
                if e % 2 == 0:
                    nc.scalar.activation(h1[:], p[:], Relu, bias=BIAS[:, e, 2:3])
                else:
                    nc.vector.tensor_scalar(out=h1[:], in0=p[:],
                                            scalar1=BIAS[:, e, 2:3], scalar2=0.0,
                                            op0=add, op1=amax)
                return h1

            def l2(j, h1a, h1b):
                # column-tiled pair: the two [128->64] matmuls target different
                # column-groups of the PE array + psum partition halves, so the
                # hardware runs them concurrently (~1 matmul's time for both)
                p = ps.tile([128, TB], f32, tag="big", bufs=6, name=f"ps2_{j}")
                nc.tensor.matmul(p[0:64, :], W["WE2"][:, j, 0, :], h1a[:],
                                 start=True, stop=True)
                nc.tensor.matmul(p[64:128, :], W["WE2"][:, j, 1, :], h1b[:],
                                 start=True, stop=True)
                h2 = sb.tile([128, TB], bf16, tag="h2", bufs=6, name=f"h2_{j}")
                if j % 2 == 0:
                    nc.scalar.activation(h2[:], p[:], Relu, bias=BIAS[:, j, 3:4])
                else:
                    nc.vector.tensor_scalar(out=h2[:], in0=p[:],
                                            scalar1=BIAS[:, j, 3:4], scalar2=0.0,
                                            op0=add, op1=amax)
                return h2

            def emit_combine(state):
                h2s, expg, bs = state
                Z = ps.tile([E, TB], f32, tag="zs", bufs=1, name="Z")
                for j in range(NPAIR):
                    nc.tensor.matmul(Z[:], W["WGB"][:, 1 + j, :], h2s[j][:],
                                     start=(j == 0), stop=(j == NPAIR - 1))
                gz = sb.tile([E, TB], f32, tag="gz", bufs=2, name="gz")
                nc.vector.tensor_mul(gz[:], Z[:], expg[:])
                nc.gpsimd.dma_start(GZd[:, bs:bs + TB], gz[:])
                nc.gpsimd.dma_start(EGd[:, bs:bs + TB], expg[:])

            state = None
            for t in range(NT):
                bs = t * TB
                xt = sb.tile([128, KD, TB], bf16, tag="xt", bufs=3, name=f"xt{t}")
                nc.sync.dma_start(xt[:], xTd[:, t])

                # gating L1
                ps_g = ps.tile([128, TB], f32, tag="psg", bufs=1, name="ps_g")
                for c in range(KD):
                    nc.tensor.matmul(ps_g[:], W["WG1"][:, c, :], xt[:, c, :],
                                     start=(c == 0), stop=(c == KD - 1))
                gh = sb.tile([128, TB], bf16, tag="gh", bufs=2, name="gh")
                nc.scalar.activation(gh[:], ps_g[:], Relu, bias=BIAS[:, 4, 3:4])

                h0s, h1s, h2s = {}, {}, {}
                h0s[0] = l0(0, xt)
                h0s[1] = l0(1, xt)

                # pipelined combine of the previous tile
                if state is not None:
                    emit_combine(state)

                # gating L2 + exp
                ps_l = ps.tile([E, TB], f32, tag="zs", bufs=1, name="ps_l")
                nc.tensor.matmul(ps_l[:], W["WGB"][:, 0, :], gh[:], start=True, stop=True)
                expg = sb.tile([E, TB], bf16, tag="eg", bufs=2, name="expg")
                nc.scalar.activation(expg[:], ps_l[:], Exp, bias=BIAS[0:8, 5, 3:4])

                # interleaved expert phases: producers run far ahead of consumers
                h0s[2] = l0(2, xt)
                h0s[3] = l0(3, xt)
                h1s[0] = l1(0, h0s[0])
                h1s[1] = l1(1, h0s[1])
                h0s[4] = l0(4, xt)
                h0s[5] = l0(5, xt)
                h1s[2] = l1(2, h0s[2])
                h1s[3] = l1(3, h0s[3])
                h0s[6] = l0(6, xt)
                h0s[7] = l0(7, xt)
                h1s[4] = l1(4, h0s[4])
                h1s[5] = l1(5, h0s[5])
                h2s[0] = l2(0, h1s[0], h1s[1])
                h2s[1] = l2(1, h1s[2], h1s[3])
                h1s[6] = l1(6, h0s[6])
                h1s[7] = l1(7, h0s[7])
                h2s[2] = l2(2, h1s[4], h1s[5])
                h2s[3] = l2(3, h1s[6], h1s[7])
                state = (h2s, expg, bs)

            emit_combine(state)

    nc.compile()
    return nc


_CACHE = {}


def _get_program():
    if "nc" not in _CACHE:
        _CACHE["nc"] = _build_program()
    return _CACHE["nc"]


def _run(inputs, trace=False):
    from concourse.bass_utils import run_bass_kernel_spmd

    x = np.ascontiguousarray(np.asarray(inputs["x"], dtype=np.float32))
    dev, ob = _fold_params(inputs)
    nc = _get_program()

    in_maps = []
    for c in range(NCORES):
        m = dict(dev)
        xs = x[c * NB:(c + 1) * NB, :].T                     # [D, NB]
        xt = xs.reshape(KD, 128, NT, TB).transpose(1, 2, 0, 3)  # [128,NT,KD,TB]
        m["xT"] = np.ascontiguousarray(xt).astype(ml_dtypes.bfloat16)
        in_maps.append(m)

    kwargs = {}
    if trace:
        kwargs = dict(trace=True, trace_cores=[0])
    res = run_bass_kernel_spmd(nc, in_maps, core_ids=list(range(NCORES)), **kwargs)
    outs = []
    for c in range(NCORES):
        gz = np.asarray(res.results[c]["GZ"], dtype=np.float64)   # [E, NB]
        eg = np.asarray(res.results[c]["EG"], dtype=np.float64)   # [E, NB]
        outs.append(gz.sum(axis=0) / eg.sum(axis=0) + ob)
    out = np.concatenate(outs)
    return out.astype(np.float32)[:, None], res


def kernel(**inputs):
    out, _ = _run(inputs, trace=False)
    return out


def kernel_traced(**inputs):
    return _run(inputs, trace=True)


# revision 13
# speedup vs baseline: 1.0162x; 1.0162x over previous
"""MoE (gating + 8 experts, BN-folded) Trainium2 Bass kernel.

Contract: kernel(**inputs) takes the FULL unsharded inputs (numpy, keyed as in
setup_inputs()) and returns the FULL [65536, 1] float32 output.

Strategy:
  * Data-parallel over 8 NeuronCores: batch 65536 -> 8192 rows per core.
  * All BatchNorms are eval-mode affine maps -> folded into the adjacent
    Linear weights/biases on the host (cheap: params < 2 MB).
  * Activations live on-chip as [features(partitions), batch(free)]; x is
    transposed + pre-tiled host-side per shard so each batch tile is one
    contiguous DMA.
  * All matmuls run bf16 (stationary weights + moving activations).
  * Per 512-row tile the PE stream is phase-ordered (all expert L0 blocks,
    then L1, then L2, interleaved) so every matmul's input activation is
    produced ~2us before the PE reaches it - no PE stalls on Act/DVE.
  * Combine: four zero-padded [128->8] projection matmuls accumulate the
    per-expert output scalars z_e into one PSUM row-block Z[8,TB]; one DVE
    multiply forms gz = exp(logits) * z.  gz and exp(logits) are DMA'd out
    and the host finishes y = sum_e gz / sum_e eg + ob (softmax divide).
  * The combine of tile t is emitted inside tile t+1's matmul stream so its
    matmuls never wait on tile t's last activations.
  * Const DMAs are batched (10 transfers) and ordered by first use so the
    PE starts ~1.5us in and warms up quickly.
"""

import numpy as np
import ml_dtypes

EPS = 1e-5
B, D, E, G, H0, H1, H2 = 65536, 256, 8, 128, 256, 128, 64
NCORES = 8
NB = B // NCORES          # rows per core
TB = 512                  # batch tile (matmul free dim / PSUM bank)
NT = NB // TB             # batch tiles per core
KD = D // 128             # k-chunks over D
NPAIR = E // 2


def _fold_params(inputs):
    """Fold the four BatchNorms into the adjacent Linears. float64 math."""
    f = {k: np.asarray(v, dtype=np.float64) for k, v in inputs.items()}

    s_in = f["in_g"] / np.sqrt(f["in_v"] + EPS)            # [D]
    t_in = f["in_b"] - f["in_m"] * s_in                    # [D]

    # gating L1 (+input BN folded in)
    a_g = f["g_g"] / np.sqrt(f["g_v"] + EPS)               # [G]
    w1 = f["gW1"] * a_g[None, :]                           # [D,G]
    W1f = s_in[:, None] * w1
    b1f = t_in @ w1 + (f["gb1"] - f["g_m"]) * a_g + f["g_b"]

    # expert L0 (+input BN)
    a0 = f["e0g"] / np.sqrt(f["e0v"] + EPS)                # [E,H0]
    w0 = f["eW0"] * a0[:, None, :]                         # [E,D,H0]
    W0f = s_in[None, :, None] * w0
    b0f = np.einsum("d,edo->eo", t_in, w0) + (f["eb0"] - f["e0m"]) * a0 + f["e0b"]

    a1 = f["e1g"] / np.sqrt(f["e1v"] + EPS)
    W1ef = f["eW1"] * a1[:, None, :]                       # [E,H0,H1]
    b1ef = (f["eb1"] - f["e1m"]) * a1 + f["e1b"]

    a2 = f["e2g"] / np.sqrt(f["e2v"] + EPS)
    W2f = f["eW2"] * a2[:, None, :]                        # [E,H1,H2]
    b2f = (f["eb2"] - f["e2m"]) * a2 + f["e2b"]

    bf = lambda a: np.ascontiguousarray(a).astype(ml_dtypes.bfloat16)

    dev = {}
    dev["WG1"] = bf(W1f.reshape(KD, 128, G).transpose(1, 0, 2))          # [128,KD,G]
    dev["WE0"] = bf(W0f.reshape(E, KD, 128, 2, 128).transpose(2, 0, 1, 3, 4))  # [128,E,KD,2,128]
    dev["WE1"] = bf(W1ef.reshape(E, 2, 128, H1).transpose(2, 0, 1, 3))   # [128,E,2,H1]
    # dense [H1,H2] per expert; pair members go to psum partition halves
    # via column-tiled concurrent matmuls
    WE2 = np.zeros((128, NPAIR, 2, H2), dtype=np.float64)
    for j in range(NPAIR):
        WE2[:, j, 0, :] = W2f[2 * j]                       # -> psum parts 0:64
        WE2[:, j, 1, :] = W2f[2 * j + 1]                   # -> psum parts 64:128
    dev["WE2"] = bf(WE2)

    # WGB: [:,0,:] gating L2 weight; [:,1+j,:] zero-padded pair projections
    ow = f["oW"][:, 0]                                     # [H2]
    WGB = np.zeros((128, 1 + NPAIR, E), dtype=np.float64)
    WGB[:, 0, :] = f["gW2"]
    for j in range(NPAIR):
        WGB[0:64, 1 + j, 2 * j] = ow
        WGB[64:128, 1 + j, 2 * j + 1] = ow
    dev["WGB"] = bf(WGB)

    # BIAS blob [128, 8, 4] f32:
    #  [:,e,0] / [:,e,1]  expert L0 bias chunks; [:,e,2] expert L1 bias
    #  [:,j,3] (j<4) paired L2 bias; [:,4,3] gating L1 bias; [0:8,5,3] gating L2 bias
    BIAS = np.zeros((128, E, 4), dtype=np.float64)
    for e in range(E):
        BIAS[:, e, 0] = b0f[e, 0:128]
        BIAS[:, e, 1] = b0f[e, 128:256]
        BIAS[:, e, 2] = b1ef[e]
    for j in range(NPAIR):
        BIAS[0:64, j, 3] = b2f[2 * j]
        BIAS[64:128, j, 3] = b2f[2 * j + 1]
    BIAS[:, 4, 3] = b1f
    BIAS[0:8, 5, 3] = f["gb2"]
    dev["BIAS"] = np.ascontiguousarray(BIAS, dtype=np.float32)

    ob = float(f["ob"][0])
    return dev, ob


def _build_program():
    import concourse.mybir as mybir
    import concourse.tile as tile
    from concourse import bacc

    f32 = mybir.dt.float32
    bf16 = mybir.dt.bfloat16
    Relu = mybir.ActivationFunctionType.Relu
    Exp = mybir.ActivationFunctionType.Exp
    add = mybir.AluOpType.add
    amax = mybir.AluOpType.max

    nc = bacc.Bacc("TRN2", target_bir_lowering=False, debug=False)

    xTd = nc.dram_tensor("xT", [128, NT, KD, TB], bf16, kind="ExternalInput").ap()
    GZd = nc.dram_tensor("GZ", [E, NB], f32, kind="ExternalOutput").ap()
    EGd = nc.dram_tensor("EG", [E, NB], bf16, kind="ExternalOutput").ap()
    d_in = {}
    for name, shape, dt in [
        ("WG1", [128, KD, G], bf16),
        ("WE0", [128, E, KD, 2, 128], bf16),
        ("WE1", [128, E, 2, H1], bf16),
        ("WE2", [128, NPAIR, 2, H2], bf16),
        ("WGB", [128, 1 + NPAIR, E], bf16),
        ("BIAS", [128, E, 4], f32),
    ]:
        d_in[name] = nc.dram_tensor(name, shape, dt, kind="ExternalInput").ap()

    with tile.TileContext(nc) as tc:
        with (
            tc.tile_pool(name="consts", bufs=1) as consts,
            tc.tile_pool(name="sb", bufs=1) as sb,
            tc.tile_pool(name="ps", bufs=1, space="PSUM") as ps,
        ):
            W = {}
            for name, ap in d_in.items():
                W[name] = consts.tile(list(ap.shape), ap.dtype, tag=name, name=name)
            # const DMAs spread across the DMA-capable queues (gpsimd/scalar;
            # sync carries the x tiles) so the initial ~0.7us-each issue ops
            # land in parallel instead of serializing on one queue
            nc.scalar.dma_start(W["WG1"][:], d_in["WG1"][:])
            nc.gpsimd.dma_start(W["WE0"][:, 0:2], d_in["WE0"][:, 0:2])
            nc.scalar.dma_start(W["BIAS"][:], d_in["BIAS"][:])
            nc.scalar.dma_start(W["WGB"][:], d_in["WGB"][:])
            nc.gpsimd.dma_start(W["WE0"][:, 2:4], d_in["WE0"][:, 2:4])
            nc.scalar.dma_start(W["WE0"][:, 4:6], d_in["WE0"][:, 4:6])
            nc.gpsimd.dma_start(W["WE0"][:, 6:8], d_in["WE0"][:, 6:8])
            nc.gpsimd.dma_start(W["WE1"][:, 0:4], d_in["WE1"][:, 0:4])
            nc.gpsimd.dma_start(W["WE1"][:, 4:8], d_in["WE1"][:, 4:8])
            nc.gpsimd.dma_start(W["WE2"][:], d_in["WE2"][:])

            BIAS = W["BIAS"]

            def l0(e, xt):
                h0 = sb.tile([128, 2, TB], bf16, tag="h0", bufs=6, name=f"h0_{e}")
                for mc in (0, 1):
                    p = ps.tile([128, TB], f32, tag="l0", bufs=4, name=f"ps0_{e}_{mc}")
                    for c in range(KD):
                        nc.tensor.matmul(p[:], W["WE0"][:, e, c, mc, :], xt[:, c, :],
                                         start=(c == 0), stop=(c == KD - 1))
                    if mc == 0:
                        nc.scalar.activation(h0[:, 0, :], p[:], Relu,
                                             bias=BIAS[:, e, 0:1])
                    else:
                        nc.vector.tensor_scalar(out=h0[:, 1, :], in0=p[:],
                                                scalar1=BIAS[:, e, 1:2], scalar2=0.0,
                                                op0=add, op1=amax)
                return h0

            def l1(e, h0):
                p = ps.tile([128, TB], f32, tag="l1", bufs=2, name=f"ps1_{e}")
                for c in (0, 1):
                    nc.tensor.matmul(p[:], W["WE1"][:, e, c, :], h0[:, c, :],
                                     start=(c == 0), stop=(c == 1))
                h1 = sb.tile([128, TB], bf16, tag="h1", bufs=9, name=f"h1_{e}")
                if e % 2 == 0:
                    nc.scalar.activation(h1[:], p[:], Relu, bias=BIAS[:, e, 2:3])
                else:
                    nc.vector.tensor_scalar(out=h1[:], in0=p[:],
                                            scalar1=BIAS[:, e, 2:3], scalar2=0.0,
                                            op0=add, op1=amax)
                return h1

            def l2(j, h1a, h1b):
                # column-tiled pair: the two [128->64] matmuls target different
                # column-groups of the PE array + psum partition halves, so the
                # hardware runs them concurrently (~1 matmul's time for both)
                p = ps.tile([128, TB], f32, tag="l0", bufs=4, name=f"ps2_{j}")
                nc.tensor.matmul(p[0:64, :], W["WE2"][:, j, 0, :], h1a[:],
                                 start=True, stop=True)
                nc.tensor.matmul(p[64:128, :], W["WE2"][:, j, 1, :], h1b[:],
                                 start=True, stop=True)
                h2 = sb.tile([128, TB], bf16, tag="h2", bufs=6, name=f"h2_{j}")
                if j % 2 == 0:
                    nc.scalar.activation(h2[:], p[:], Relu, bias=BIAS[:, j, 3:4])
                else:
                    nc.vector.tensor_scalar(out=h2[:], in0=p[:],
                                            scalar1=BIAS[:, j, 3:4], scalar2=0.0,
                                            op0=add, op1=amax)
                return h2

            def emit_combine(state):
                h2s, expg, bs = state
                Z = ps.tile([E, TB], f32, tag="zs", bufs=2, name="Z")
                for j in range(NPAIR):
                    nc.tensor.matmul(Z[:], W["WGB"][:, 1 + j, :], h2s[j][:],
                                     start=(j == 0), stop=(j == NPAIR - 1))
                gz = sb.tile([E, TB], f32, tag="gz", bufs=2, name="gz")
                nc.vector.tensor_mul(gz[:], Z[:], expg[:])
                nc.gpsimd.dma_start(GZd[:, bs:bs + TB], gz[:])
                nc.gpsimd.dma_start(EGd[:, bs:bs + TB], expg[:])

            state = None
            for t in range(NT):
                bs = t * TB
                xt = sb.tile([128, KD, TB], bf16, tag="xt", bufs=3, name=f"xt{t}")
                nc.sync.dma_start(xt[:], xTd[:, t])

                # gating L1
                ps_g = ps.tile([128, TB], f32, tag="l1", bufs=2, name="ps_g")
                for c in range(KD):
                    nc.tensor.matmul(ps_g[:], W["WG1"][:, c, :], xt[:, c, :],
                                     start=(c == 0), stop=(c == KD - 1))
                gh = sb.tile([128, TB], bf16, tag="gh", bufs=2, name="gh")
                nc.scalar.activation(gh[:], ps_g[:], Relu, bias=BIAS[:, 4, 3:4])

                h0s, h1s, h2s = {}, {}, {}
                h0s[0] = l0(0, xt)
                h0s[1] = l0(1, xt)

                # pipelined combine of the previous tile
                if state is not None:
                    emit_combine(state)

                # gating L2 + exp
                ps_l = ps.tile([E, TB], f32, tag="zs", bufs=2, name="ps_l")
                nc.tensor.matmul(ps_l[:], W["WGB"][:, 0, :], gh[:], start=True, stop=True)
                expg = sb.tile([E, TB], bf16, tag="eg", bufs=2, name="expg")
                nc.scalar.activation(expg[:], ps_l[:], Exp, bias=BIAS[0:8, 5, 3:4])

                # interleaved expert phases: producers run far ahead of consumers
                h0s[2] = l0(2, xt)
                h0s[3] = l0(3, xt)
                h1s[0] = l1(0, h0s[0])
                h1s[1] = l1(1, h0s[1])
                h0s[4] = l0(4, xt)
                h0s[5] = l0(5, xt)
                h1s[2] = l1(2, h0s[2])
                h1s[3] = l1(3, h0s[3])
                h0s[6] = l0(6, xt)
                h0s[7] = l0(7, xt)
                h1s[4] = l1(4, h0s[4])
                h1s[5] = l1(5, h0s[5])
                h2s[0] = l2(0, h1s[0], h1s[1])
                h2s[1] = l2(1, h1s[2], h1s[3])
                h1s[6] = l1(6, h0s[6])
                h1s[7] = l1(7, h0s[7])
                h2s[2] = l2(2, h1s[4], h1s[5])
                h2s[3] = l2(3, h1s[6], h1s[7])
                state = (h2s, expg, bs)

            emit_combine(state)

    nc.compile()
    return nc


_CACHE = {}


def _get_program():
    if "nc" not in _CACHE:
        _CACHE["nc"] = _build_program()
    return _CACHE["nc"]


def _run(inputs, trace=False):
    from concourse.bass_utils import run_bass_kernel_spmd

    x = np.ascontiguousarray(np.asarray(inputs["x"], dtype=np.float32))
    dev, ob = _fold_params(inputs)
    nc = _get_program()

    in_maps = []
    for c in range(NCORES):
        m = dict(dev)
        xs = x[c * NB:(c + 1) * NB, :].T                     # [D, NB]
        xt = xs.reshape(KD, 128, NT, TB).transpose(1, 2, 0, 3)  # [128,NT,KD,TB]
        m["xT"] = np.ascontiguousarray(xt).astype(ml_dtypes.bfloat16)
        in_maps.append(m)

    kwargs = {}
    if trace:
        kwargs = dict(trace=True, trace_cores=[0])
    res = run_bass_kernel_spmd(nc, in_maps, core_ids=list(range(NCORES)), **kwargs)
    outs = []
    for c in range(NCORES):
        gz = np.asarray(res.results[c]["GZ"], dtype=np.float64)   # [E, NB]
        eg = np.asarray(res.results[c]["EG"], dtype=np.float64)   # [E, NB]
        outs.append(gz.sum(axis=0) / eg.sum(axis=0) + ob)
    out = np.concatenate(outs)
    return out.astype(np.float32)[:, None], res


def kernel(**inputs):
    out, _ = _run(inputs, trace=False)
    return out


def kernel_traced(**inputs):
    return _run(inputs, trace=True)
